# revision 42
# baseline (speedup 1.0000x reference)
"""GAT (2-layer, 8-head) Bass kernel for 8 Trainium2 NeuronCores.

Strategy (row-sharded attention):
  - Core d owns 512 rows (nodes) i in [512d, 512d+512).
  - Layer-1: each core computes h_k = x_d @ W_k for its rows (all 8 heads),
    plus score vectors; AllGather waves share [v_j*h_j | v_j | z_j] for all
    nodes; each core then computes its row-block of masked attention and
    h'_k via PE matmuls, using the separable-exponential decomposition
        exp(leakyrelu(s1_i + s2_j)) = u_i * v_j * max(w_i * z_j, 1)
    with u = exp(.2 s1) (cancels in softmax), v = exp(.2 s2),
    w = exp(.8 s1), z = exp(.8 s2).
    The unnormalized masked score matrix (transposed, [j,i]-layout) is
        U[j,i] = max(Wb[j,i] * z_j, 1) * mask[j,i]      (v folded into h)
    built with one dual-scalar TENSOR_SCALAR (DVE) or one RELU (ACT, with
    a PE mask-matmul recovering the "+1") per tile, then one TENSOR_TENSOR
    (DVE or Pool) applying the mask per 4-tile group; contracted on the PE
    against [v*h | v] to get numerator and denominator in one PSUM
    accumulation.
  - Layer-2: same pipeline once more on z = mean_k elu(h'_k).

AllGather buffers are partition-major ([P, ...] rows) so both the payload
write and the gathered read are large contiguous per-partition DMA
descriptors. The mask (adj > 0) is pre-transposed and bf16-encoded on the
host and loaded on queues away from the payload path.
"""

import numpy as np
import ml_dtypes

import concourse.bass as bass
import concourse.bacc as bacc
import concourse.tile as tile
import concourse.mybir as mybir
from concourse.bass_utils import run_bass_kernel_spmd
from concourse.masks import make_identity

dt = mybir.dt
Alu = mybir.AluOpType
Act = mybir.ActivationFunctionType
AX = mybir.AxisListType

NCORES = 8
N, F, NH, KH, NO = 4096, 512, 64, 8, 56
P = 128
R = N // NCORES          # rows per core = 512
IT = R // P              # i-tiles per core = 4
NB = N // P              # j-blocks = 32
GRP = 4                  # j-blocks per TT group (= IT, so group == core)
NG = NB // GRP           # groups = 8
C1 = NH + 2              # layer-1 payload cols [v*h | v | z]
C2 = NO + 2              # layer-2 payload cols
bf16 = ml_dtypes.bfloat16

# phase-B tuning: per-group count of ACT-produced tiles, Pool TT groups
# (Pool TT disabled: concurrent Pool SBUF traffic slows DVE ops ~4x)
NACT_PAT = (2, 2, 2, 2, 2, 2, 2, 2)
POOL_GROUPS = ()

_CACHE: dict = {}


def _build():
    nc = bacc.Bacc("TRN2", target_bir_lowering=False, debug=False,
                   num_devices=NCORES)

    # ---- I/O -----------------------------------------------------------
    xT_d = nc.dram_tensor("xT", [P, 4, R], dt.bfloat16, kind="ExternalInput")
    maskT_d = nc.dram_tensor("maskT", [P, NB, R], dt.bfloat16,
                             kind="ExternalInput")
    wkt_d = nc.dram_tensor("wkt", [P, KH // 2, 4, P], dt.bfloat16,
                           kind="ExternalInput")
    waal2_d = nc.dram_tensor("waal2", [P, 4, KH], dt.bfloat16,
                             kind="ExternalInput")
    waball_d = nc.dram_tensor("waball", [P, 4, 2, 97], dt.bfloat16,
                              kind="ExternalInput")
    wout_d = nc.dram_tensor("wout", [NH, NO], dt.bfloat16,
                            kind="ExternalInput")
    wa12_d = nc.dram_tensor("wa12", [NH, 2], dt.bfloat16,
                            kind="ExternalInput")
    out_d = nc.dram_tensor("out", [R, NO], dt.float32, kind="ExternalOutput")

    with tile.TileContext(nc) as tc:
        _emit(nc, tc, xT_d, maskT_d, wkt_d, waal2_d, waball_d, wout_d,
              wa12_d, out_d)

    nc.compile()
    return nc


def _emit(nc, tc, xT_d, maskT_d, wkt_d, waal2_d, waball_d, wout_d, wa12_d,
          out_d):
    from contextlib import ExitStack
    ctx = ExitStack()
    with ctx:
        const = ctx.enter_context(tc.tile_pool(name="const", bufs=1))
        dram = ctx.enter_context(tc.tile_pool(name="dram", bufs=1,
                                              space="DRAM"))
        pa = ctx.enter_context(tc.tile_pool(name="pa", bufs=1, space="PSUM"))
        pb = ctx.enter_context(tc.tile_pool(name="pb", bufs=2, space="PSUM"))
        pt = ctx.enter_context(tc.tile_pool(name="pt", bufs=1, space="PSUM"))
        sp = ctx.enter_context(tc.tile_pool(name="sp", bufs=3))
        wp = ctx.enter_context(tc.tile_pool(name="wp", bufs=8))
        hp = ctx.enter_context(tc.tile_pool(name="hp", bufs=2))
        pp2 = ctx.enter_context(tc.tile_pool(name="pp2", bufs=4))
        tp = ctx.enter_context(tc.tile_pool(name="tp", bufs=3))
        up = ctx.enter_context(tc.tile_pool(name="up", bufs=3))
        bp = ctx.enter_context(tc.tile_pool(name="bp", bufs=1))
        cp = ctx.enter_context(tc.tile_pool(name="cp", bufs=4))
        zp = ctx.enter_context(tc.tile_pool(name="zp", bufs=1))

        # ---- resident loads: payload-path tensors first, on sync -------
        xT_sb = const.tile([P, 4, R], dt.bfloat16)
        nc.sync.dma_start(out=xT_sb, in_=xT_d.ap())
        wkp_sb = const.tile([P, KH // 2, 4, P], dt.bfloat16)
        nc.sync.dma_start(out=wkp_sb, in_=wkt_d.ap())
        waal2_sb = const.tile([P, 4, KH], dt.bfloat16)
        nc.sync.dma_start(out=waal2_sb, in_=waal2_d.ap())
        waball_sb = const.tile([P, 4, 2, 97], dt.bfloat16)
        nc.sync.dma_start(out=waball_sb, in_=waball_d.ap())
        wout_sb = const.tile([NH, NO], dt.bfloat16)
        nc.sync.dma_start(out=wout_sb, in_=wout_d.ap())
        wa12_sb = const.tile([NH, 2], dt.bfloat16)
        nc.sync.dma_start(out=wa12_sb, in_=wa12_d.ap())
        # mask loads ride the gpsimd queue in chunks; the first-collective
        # floor (~70us) means payload-trigger time is not critical, only
        # that the mask never starves the weight loads at t=0
        mask_sb = const.tile([P, NB, R], dt.bfloat16)

        def _mask_chunk(c):
            lo, hi = 4 * c, 4 * (c + 1)
            nc.gpsimd.dma_start(out=mask_sb[:, lo:hi, :],
                                in_=maskT_d.ap()[:, lo:hi, :])



        idb = const.tile([P, P], dt.bfloat16)
        make_identity(nc, idb)
        idf = const.tile([P, P], dt.float32)
        make_identity(nc, idf)
        neg1 = const.tile([P, 1], dt.float32)
        nc.vector.memset(neg1, -1.0)
        ln8 = const.tile([P, 1], dt.float32)
        nc.vector.memset(ln8, float(np.log(0.125)))
        ones1 = const.tile([1, P], dt.bfloat16)
        nc.vector.memset(ones1, 1.0)

        # persistent accumulators
        u_all = zp.tile([P, IT, KH, NH], dt.float32)   # h'_k per head (divided)

        AG_SPLIT = [1, 1, 2, 4]                    # heads per gather wave
        AG_BASE = [0, 1, 2, 4]
        WAVE_OF = [0, 1, 2, 2, 3, 3, 3, 3]
        ag_ins, ag_outs = [], []
        for nk in AG_SPLIT:
            ag_ins.append(dram.tile([P, nk, IT, C1], dt.bfloat16,
                                    name=f"agi{len(ag_ins)}"))
            ag_outs.append(dram.tile([NCORES * P, nk, IT, C1], dt.bfloat16,
                                     addr_space="Shared",
                                     name=f"ago{len(ag_outs)}"))
        ag2_in = dram.tile([P, IT, C2], dt.bfloat16)
        ag2_out = dram.tile([NCORES * P, IT, C2], dt.bfloat16,
                            addr_space="Shared")

        # ================= phase A: batched h + payload =================
        # h in natural [i, hid] layout (x^T slices stationary, W streamed);
        # s2-columns for all 8 heads from one host-precomputed (W @ a2)
        # stationary; w-rows (exp .8 s1) for all heads from one small
        # (W @ a1) matmul set.
        # sall (cols 0:8) and zsum (cols 8:72) share one PSUM bank
        ps_misc = pa.tile([P, IT, KH + NH], dt.float32, tag="sall")
        ps_sall = ps_misc[:, :, 0:KH]
        scl_sb = sp.tile([P, IT, KH], dt.bfloat16, tag="scl", bufs=1)
        # s1-rows for 4 heads at a time, packed at partition stride 32 so
        # the ACT exp reads are base-partition aligned
        wrows = []
        for t in range(2):
            ps_s4 = pa.tile([P, R], dt.float32, tag="wb")
            for fb in range(4):
                nc.tensor.matmul(ps_s4[0:97, :], waball_sb[:, fb, t, :],
                                 xT_sb[:, fb, :],
                                 start=(fb == 0), stop=(fb == 3))
            for j in range(4):
                k = 4 * t + j
                wr = sp.tile([1, R], dt.bfloat16, tag="wrow8", bufs=8,
                             name=f"wrow{k}")
                nc.scalar.activation(wr, ps_s4[32 * j:32 * j + 1, :],
                                     Act.Exp, scale=0.8)
                wrows.append(wr)

        def hn_batch(half):
            prs = [2 * half, 2 * half + 1]
            ps_hns = {pr: pa.tile([P, IT, P], dt.float32, tag="hn", bufs=2,
                                  name=f"ps_hn{pr}")
                      for pr in prs}
            for isl in range(IT):
                sl = slice(isl * P, (isl + 1) * P)
                for fb in range(4):
                    lhsT = xT_sb[:, fb, sl]
                    for pr in prs:
                        nc.tensor.matmul(ps_hns[pr][:, isl, :], lhsT,
                                         wkp_sb[:, pr, fb, :],
                                         start=(fb == 0), stop=(fb == 3))
                    if half == 0:
                        nc.tensor.matmul(ps_sall[:, isl, :], lhsT,
                                         waal2_sb[:, fb, :],
                                         start=(fb == 0), stop=(fb == 3))
                if half == 0:
                    nc.any.tensor_copy(scl_sb[:, isl, :], ps_sall[:, isl, :])
            return ps_hns

        def pair_payload(pr, ps_hn):
            Wbs = []
            for hh in range(2):
                k = 2 * pr + hh
                ps_wb = pa.tile([P, R], dt.float32, tag="wb")
                nc.tensor.matmul(ps_wb, ones1, wrows[k],
                                 start=True, stop=True)
                Wb = wp.tile([P, R], dt.bfloat16, tag="Wb")
                nc.vector.tensor_copy(Wb, ps_wb)
                Wbs.append(Wb)
                vcol = sp.tile([P, IT], dt.float32, tag="vcol")
                nc.scalar.activation(vcol, scl_sb[:, :, k], Act.Exp,
                                     scale=0.2)
                pay = pp2.tile([P, IT, C1], dt.bfloat16, tag="pay")
                for isl in range(IT):
                    nc.vector.tensor_scalar(
                        pay[:, isl, 0:NH],
                        ps_hn[:, isl, hh * NH:(hh + 1) * NH],
                        vcol[:, isl:isl + 1], None, Alu.mult)
                nc.vector.tensor_copy(pay[:, :, NH], vcol)
                nc.scalar.activation(pay[:, :, NH + 1], scl_sb[:, :, k],
                                     Act.Exp, scale=0.8)
                wave = WAVE_OF[k]
                agi, kk = ag_ins[wave], k - AG_BASE[wave]
                nc.sync.dma_start(out=agi[:, kk, :, :], in_=pay)
                pair_payload.last_pay = pay
                if k == AG_BASE[wave] + AG_SPLIT[wave] - 1:
                    nc.gpsimd.collective_compute(
                        "AllGather", Alu.bypass,
                        ins=[ag_ins[wave].opt()],
                        outs=[ag_outs[wave].opt()],
                        replica_groups=[list(range(NCORES))])
            return Wbs

        _mask_chunk(0)
        _mask_chunk(1)
        Wb_k = []
        for half in range(2):
            ps_hns = hn_batch(half)
            for pr in (2 * half, 2 * half + 1):
                Wb_k += pair_payload(pr, ps_hns[pr])
                if pr < 3:
                    _mask_chunk(2 + 2 * pr)
                    _mask_chunk(3 + 2 * pr)

        # gathered per-head loads (sync queue: after payload writes)
        hsb_heads = []
        for k in range(KH):
            wv = WAVE_OF[k]
            kk = k - AG_BASE[wv]
            hk = bp.tile([P, NCORES, IT, C1], dt.bfloat16, name=f"hsbk{k}")
            src = ag_outs[wv][:, kk, :, :].rearrange(
                "(core p) isl c -> p core isl c", p=P)
            nc.sync.dma_start(out=hk, in_=src)
            hsb_heads.append(hk)

        # ================= phase B: per-head attention ==================
        def attn_units(Wb, ps_nm, triples, grp, nact, first, last, ttag):
            """triples: (stationary_ap, z_scalar_ap, jb) in order; grp
            consecutive-jb blocks share one mask TT; first nact of each
            group are ACT-produced (relu(z*w-1), '+1' recovered by an
            extra mask matmul)."""
            n = len(triples)
            for g0 in range(0, n, grp):
                seg = triples[g0:g0 + grp]
                Tg = tp.tile([P, grp, R], dt.bfloat16, tag=ttag)
                for q, (stat, zsc, jb) in enumerate(seg):
                    if q < nact:
                        nc.scalar.activation(Tg[:, q, :], Wb, Act.Relu,
                                             bias=neg1[:, 0:1], scale=zsc)
                    else:
                        nc.vector.tensor_scalar(Tg[:, q, :], Wb, zsc, 1.0,
                                                Alu.mult, Alu.max)
                jb0 = seg[0][2]
                Ug = up.tile([P, grp, R], dt.bfloat16, tag=ttag + "u")
                nc.vector.tensor_tensor(Ug, Tg,
                                        mask_sb[:, jb0:jb0 + grp, :],
                                        Alu.mult)
                for q, (stat, zsc, jb) in enumerate(seg):
                    st = first and g0 == 0 and q == 0
                    sp_ = last and g0 + grp >= n and q == len(seg) - 1
                    nc.tensor.matmul(ps_nm, stat, Ug[:, q, :],
                                     start=st, stop=sp_)
                    if q < nact:
                        nc.tensor.matmul(ps_nm, stat, mask_sb[:, jb, :],
                                         start=False, stop=False)

        def finish_unit(ps_nm, ncols):
            nmf = sp.tile([ncols, R], dt.float32, tag="nmf")
            nc.any.tensor_copy(nmf, ps_nm)
            ps_t = pt.tile([P, IT, ncols], dt.float32, tag="tr")
            for isl in range(IT):
                sl = slice(isl * P, (isl + 1) * P)
                nc.tensor.transpose(ps_t[:, isl, :], nmf[:, sl],
                                    idf[0:ncols, 0:ncols])
            return ps_t

        # phase C is folded into the head loop: per head, ACT computes the
        # two elu terms 0.125*relu(u) and 0.125*exp(min(u,0)) and the PE
        # accumulates them into ps_zsum via identity passes, so only
        # head 7's ~2us of work remains after the last attention unit.
        ps_zsum = ps_misc[:, :, KH:KH + NH]
        for k in range(KH):
            hsb5 = hsb_heads[k]
            zf = sp.tile([P, NG, IT], dt.float32, tag="zf")
            nc.vector.tensor_copy(zf, hsb5[:, :, :, NH + 1])
            ps_nm = pb.tile([NH + 1, R], dt.float32, tag="nm")
            triples = [(hsb5[:, jb // IT, jb % IT, 0:NH + 1],
                        zf[:, jb // IT, jb % IT:jb % IT + 1], jb)
                       for jb in range(NB)]
            attn_units(Wb_k[k], ps_nm, triples, GRP, 2, True, True, "T")
            ps_t = finish_unit(ps_nm, NH + 1)
            rc4 = sp.tile([P, IT], dt.float32, tag="rc")
            nc.vector.reciprocal(rc4, ps_t[:, :, NH])
            for isl in range(IT):
                nc.scalar.activation(u_all[:, isl, k, :],
                                     ps_t[:, isl, 0:NH], Act.Copy,
                                     scale=rc4[:, isl:isl + 1])
            rn = cp.tile([P, IT, NH], dt.float32, tag="rn")
            nc.scalar.activation(rn, u_all[:, :, k, :], Act.Relu,
                                 scale=-1.0)
            Bpp = cp.tile([P, IT, NH], dt.bfloat16, tag="Bp")
            nc.scalar.activation(Bpp, rn, Act.Exp, bias=ln8[:, 0:1],
                                 scale=-1.0)
            Dpp = cp.tile([P, IT, NH], dt.bfloat16, tag="Dp")
            nc.scalar.activation(Dpp, u_all[:, :, k, :], Act.Relu,
                                 scale=0.125)
            nc.tensor.matmul(ps_zsum, idb, Dpp,
                             start=(k == 0), stop=False)
            nc.tensor.matmul(ps_zsum, idb, Bpp,
                             start=False, stop=(k == KH - 1))

        # ================= phase D: layer-2 h2 + payload (2 halves) =====
        zbf = zp.tile([P, IT, NH], dt.bfloat16)
        ps_zT = pa.tile([NH, R], dt.bfloat16, tag="wb")
        zT = hp.tile([NH, R], dt.bfloat16, tag="hT_s")
        h2T = hp.tile([NO, R], dt.bfloat16, tag="h2T_s")
        s2row = sp.tile([2, R], dt.bfloat16, tag="srow_s", bufs=1)
        w2row = sp.tile([1, R], dt.bfloat16, tag="w2row", bufs=1)
        ps_wb2 = pa.tile([P, R], dt.float32, tag="hn", bufs=2)
        Wb2 = wp.tile([P, R], dt.bfloat16, tag="Wb")
        ps_h2n = pa.tile([P, IT, C2], dt.bfloat16, tag="hn", bufs=2)
        v2col = sp.tile([P, IT], dt.float32, tag="v2col", bufs=1)
        for h in range(2):
            hs = slice(h * 256, (h + 1) * 256)
            nc.vector.tensor_scalar(zbf[:, 2 * h:2 * h + 2, :],
                                    ps_zsum[:, 2 * h:2 * h + 2, :],
                                    -1.0, None, Alu.add)
            for isl in (2 * h, 2 * h + 1):
                sl = slice(isl * P, (isl + 1) * P)
                nc.tensor.transpose(ps_zT[:, sl], zbf[:, isl, :], idb)
            nc.vector.tensor_copy(zT[:, hs], ps_zT[:, hs])
            ps_h2T = pa.tile([NO, 256], dt.float32, tag="srow",
                             name=f"ps_h2T{h}")
            nc.tensor.matmul(ps_h2T, wout_sb, zT[:, hs],
                             start=True, stop=True)
            nc.any.tensor_copy(h2T[:, hs], ps_h2T)
            ps_s12 = pa.tile([2, 256], dt.float32, tag="srow",
                             name=f"ps_s12{h}")
            nc.tensor.matmul(ps_s12, wa12_sb, zT[:, hs],
                             start=True, stop=True)
            nc.any.tensor_copy(s2row[:, hs], ps_s12)
            nc.scalar.activation(w2row[:, hs], s2row[0:1, hs], Act.Exp,
                                 scale=0.8)
            nc.tensor.matmul(ps_wb2[:, hs], ones1, w2row[:, hs],
                             start=True, stop=True)
            nc.vector.tensor_copy(Wb2[:, hs], ps_wb2[:, hs])
            pay2 = pp2.tile([P, 2, C2], dt.bfloat16, tag="pay2")
            for ii, isl in enumerate((2 * h, 2 * h + 1)):
                sl = slice(isl * P, (isl + 1) * P)
                nc.tensor.transpose(ps_h2n[:, isl, 0:NO], h2T[:, sl],
                                    idb[0:NO, 0:NO])
                nc.tensor.transpose(ps_h2n[:, isl, NO:NO + 2], s2row[:, sl],
                                    idb[0:2, 0:2])
            nc.scalar.activation(v2col[:, 2 * h:2 * h + 2],
                                 ps_h2n[:, 2 * h:2 * h + 2, NO + 1],
                                 Act.Exp, scale=0.2)
            for ii, isl in enumerate((2 * h, 2 * h + 1)):
                nc.vector.tensor_scalar(pay2[:, ii, 0:NO],
                                        ps_h2n[:, isl, 0:NO],
                                        v2col[:, isl:isl + 1], None,
                                        Alu.mult)
            nc.vector.tensor_copy(pay2[:, :, NO],
                                  v2col[:, 2 * h:2 * h + 2])
            nc.scalar.activation(pay2[:, :, NO + 1],
                                 ps_h2n[:, 2 * h:2 * h + 2, NO + 1],
                                 Act.Exp, scale=0.8)
            nc.sync.dma_start(out=ag2_in[:, 2 * h:2 * h + 2, :], in_=pay2)
        nc.gpsimd.collective_compute(
            "AllGather", Alu.bypass,
            ins=[ag2_in.opt()], outs=[ag2_out.opt()],
            replica_groups=[list(range(NCORES))])

        # ================= phase E: attention-2 + softmax ===============
        zf2 = sp.tile([P, NCORES, IT], dt.float32, tag="zf")
        ps_nm2 = pb.tile([NO + 1, R], dt.float32, tag="nm")
        hsb2 = bp.tile([P, NCORES, IT, C2], dt.bfloat16, name="hsb2")
        nc.sync.dma_start(
            out=hsb2,
            in_=ag2_out.rearrange("(core p) isl c -> p core isl c", p=P))
        nc.vector.tensor_copy(zf2, hsb2[:, :, :, NO + 1])
        triples = [(hsb2[:, jb // IT, jb % IT, 0:NO + 1],
                    zf2[:, jb // IT, jb % IT:jb % IT + 1], jb)
                   for jb in range(NB)]
        attn_units(Wb2, ps_nm2, triples, GRP, 2, True, True, "T")
        ps_t2 = finish_unit(ps_nm2, NO + 1)
        for isl in range(IT):
            rc = sp.tile([P, 1], dt.float32, tag="rc")
            nc.vector.reciprocal(rc, ps_t2[:, isl, NO:NO + 1])
            ue = cp.tile([P, NO], dt.float32, tag="ue")
            nc.vector.tensor_scalar(ue, ps_t2[:, isl, 0:NO], rc, None,
                                    Alu.mult)
            e2 = cp.tile([P, NO], dt.float32, tag="e2")
            nc.scalar.activation(e2, ue, Act.Exp)
            t1 = cp.tile([P, NO], dt.float32, tag="t1")
            nc.vector.tensor_scalar(t1, e2, 1.0, -1.0, Alu.min, Alu.add)
            el = cp.tile([P, NO], dt.float32, tag="el")
            nc.vector.scalar_tensor_tensor(el, ue, 0.0, t1, Alu.max, Alu.add)
            # |elu(h2')| is O(1): plain exp is overflow-safe, and the
            # max-subtraction cancels exactly in the softmax ratio
            ex = cp.tile([P, NO], dt.float32, tag="ex")
            nc.scalar.activation(ex, el, Act.Exp)
            sm = sp.tile([P, 1], dt.float32, tag="sm")
            nc.vector.tensor_reduce(sm, ex, AX.X, Alu.add)
            rc2 = sp.tile([P, 1], dt.float32, tag="rc2")
            nc.vector.reciprocal(rc2, sm)
            oo = cp.tile([P, NO], dt.float32, tag="oo")
            nc.vector.tensor_scalar(oo, ex, rc2, None, Alu.mult)
            nc.sync.dma_start(out=out_d.ap()[isl * P:(isl + 1) * P, :],
                              in_=oo)


def _prep_inputs(x, adj, Ws, As, W_out, a_out):
    x32 = np.asarray(x, np.float32)
    adj_np = np.asarray(adj)
    mask_full = adj_np > 0
    Ws32 = np.asarray(Ws, np.float32)              # [8, 512, 64]
    wkt = np.zeros((KH // 2, 4, P, P), np.float32)
    for pr in range(KH // 2):
        pairw = np.concatenate([Ws32[2 * pr], Ws32[2 * pr + 1]],
                               axis=1)              # [512, 128]
        wkt[pr] = pairw.reshape(4, P, P)
    wkt = np.ascontiguousarray(wkt.transpose(2, 0, 1, 3)).astype(bf16)
    av = np.asarray(As, np.float64)[:, :, 0]        # [8, 128]
    Ws64 = np.asarray(Ws, np.float64)
    wa = np.zeros((KH, 2, F), np.float64)           # (head, a1/a2, feat)
    for k in range(KH):
        wa[k, 0] = Ws64[k] @ av[k, :NH]
        wa[k, 1] = Ws64[k] @ av[k, NH:]
    waal2 = np.zeros((P, 4, KH), np.float32)
    waball = np.zeros((P, 4, 2, 97), np.float32)
    for k in range(KH):
        waal2[:, :, k] = wa[k, 1].reshape(4, P).T
        waball[:, :, k // 4, 32 * (k % 4)] = wa[k, 0].reshape(4, P).T
    waal2 = np.ascontiguousarray(waal2).astype(bf16)
    waball = np.ascontiguousarray(waball).astype(bf16)
    wout = np.asarray(W_out, np.float32).astype(bf16)
    Wo64 = np.asarray(W_out, np.float64)
    ao = np.asarray(a_out, np.float64)[:, 0]
    wa12 = np.ascontiguousarray(
        np.stack([Wo64 @ ao[:NO], Wo64 @ ao[NO:]], axis=-1)).astype(bf16)

    in_maps = []
    for d in range(NCORES):
        rows = slice(R * d, R * (d + 1))
        xT = np.ascontiguousarray(
            x32[rows].T.reshape(4, P, R).transpose(1, 0, 2)).astype(bf16)
        maskT = np.ascontiguousarray(
            mask_full[rows].T.astype(bf16).reshape(NB, P, R)
            .transpose(1, 0, 2))
        in_maps.append({
            "xT": xT, "maskT": maskT, "wkt": wkt, "waal2": waal2,
            "waball": waball,
            "wout": wout, "wa12": wa12,
        })
    return in_maps


def kernel(x, adj, Ws, As, W_out, a_out, trace=False):
    if "nc" not in _CACHE:
        _CACHE["nc"] = _build()
    nc = _CACHE["nc"]
    in_maps = _prep_inputs(x, adj, Ws, As, W_out, a_out)
    res = run_bass_kernel_spmd(nc, in_maps, list(range(NCORES)), trace=trace)
    out = np.concatenate([res.results[d]["out"] for d in range(NCORES)],
                         axis=0).astype(np.float32)
    if trace:
        kernel.last_exec_time_ns = res.exec_time_ns
    return out
